# revision 33
# baseline (speedup 1.0000x reference)
"""Bahdanau attention Trainium2 kernel (v2: fp8 hi/lo DoubleRow energy GEMM).

score(s, h_i) = v . tanh(W_s s + W_h h_i);  softmax over S;  context = w @ enc.

Strategy (per NeuronCore, data-parallel over batch, 8 batches/core):
  - enc loads HBM->SBUF as bf16 via SWDGE cast-DMA (one 2MiB op per
    512-row block).
  - enc_bf splits into fp8 (hi, lo) pairs packed into bf16-typed lanes:
    hi = e4m3(x), lo = e4m3(x - hi).  One 2-byte-granular transpose per
    128-row subtile (2 on xbar DMA, 2 on PE transpose-mode per block)
    yields encT[p_h, c, s] whose fp8 view [p, c, j, s] feeds DoubleRow
    matmuls directly.
  - energy GEMM (the FLOP bulk) runs in fp8 DoubleRow: per 128-h chunk
    one MM pairs (Whi, Whi)x(Ehi, Elo); per 2 chunks one cross MM pairs
    (Wlo[c], Wlo[c+1])x(Ehi[c], Ehi[c+1]) for 12 of 16 chunks (CROSS=6).
    W_h is scaled x32 into e4m3 range (compensated in the tanh scale).
    End-to-end rel err ~9e-3 (vs 2.5e-2 for naive fp8) at ~1.4 DR-MMs
    per chunk instead of 2 bf16 MMs.
  - scores = v . tanh(energyT) via bf16 matmuls; dec_proj (computed on
    device from a strided-transposed dec load) folds in as the
    per-partition ACT bias of the fused tanh (scale=1/32).
  - softmax without max-subtraction (|scores| small, exp safe in f32);
    mask-multiply and block-sum fused in one tensor_tensor_reduce.
  - context: ctxT[h-chunk] += (hi|lo)[s, h-chunk].T @ e_col via N=1
    matmuls (nearly free on PE, one PSUM bank zero-filled per batch);
    e transposes to columns via K=1 matmuls; 1/sum applied at the end.
  - the whole thing runs as a software pipeline over 512-row blocks:
    load(g+2) / cast(g) / compute(g-1) / tail(g-2), with engine
    assignment patterns tuned so the ACT tanh chain stays responsive.
"""

import sys
from contextlib import ExitStack

sys.path.insert(0, "/opt/trn_rl_repo")

import numpy as np
import ml_dtypes

import concourse.bass as bass
import concourse.tile as tile
from concourse import mybir

# ---- walrus workaround: tail drain accepts only 1 sync wait ----------------
from concourse.vector_clock import ScopedClock, VectorClock


def _patched_drain_and_barrier(self, tick_clock, wait_clock):
    gc = tick_clock.global_clock
    procs = [(i, gc[i]) for i in range(len(gc)) if gc[i] > 0]
    for p, t in procs:
        vc = VectorClock()
        vc.require_at_least(p, t)
        nop = self.nc.sync.nop(nofuse=True, hint="tail_wait_split")
        wait_clock.add_sem_waits(nop.ins, ScopedClock({None: vc}))
    self.nc.sync.drain()
    self.nc.all_engine_barrier()
    assert self.sems is not None
    popped = self.nc._tile_sem_poison_stack.pop()
    assert popped is self._sem_poison
    self.nc.clear_and_free_semaphores(list(self.sems.allocated().values()))
    self.nc.all_engine_barrier()


tile.TileContext._drain_and_barrier = _patched_drain_and_barrier


def _spill_excess_waits(nc):
    """This walrus build accepts at most 1 sync wait per instruction (2 for
    EventSemaphore).  Move the excess onto same-engine NOPs inserted
    immediately before the instruction."""
    import bass_rust

    nop_id = [0]
    for fn in nc.m.functions:
        for blk in fn.blocks:
            new_insts = []
            changed = False
            for inst in blk.instructions:
                si = inst.sync_info
                cap = 2 if type(inst).__name__ == "InstEventSemaphore" else 1
                if si is not None and len(si.on_wait) > cap:
                    waits = list(si.on_wait)
                    keep, spill = waits[-cap:], waits[:-cap]
                    for w in spill:
                        nop = mybir.InstNoOp(
                            name=f"I-waitspill-{nop_id[0]}", ins=[], outs=[]
                        )
                        nop_id[0] += 1
                        nop.engine = inst.engine
                        nop.sync_info = bass_rust.SyncInfo(
                            on_wait=[w], on_update=[]
                        )
                        nc.register_instruction(nop, overwrite=True)
                        new_insts.append(nop)
                    inst.sync_info = bass_rust.SyncInfo(
                        on_wait=keep, on_update=list(si.on_update)
                    )
                    changed = True
                new_insts.append(inst)
            if changed:
                blk.instructions = new_insts
    return nc


# ---------------------------------------------------------------------------

N_CORES = 8
B, S, H, A = 64, 2048, 1024, 512
H2 = 2 * H
F32 = mybir.dt.float32
BF16 = mybir.dt.bfloat16
FP8 = mybir.dt.float8e4
U16 = mybir.dt.uint16
U8 = mybir.dt.uint8
AF = mybir.ActivationFunctionType
ALU = mybir.AluOpType
DR = mybir.MatmulPerfMode.DoubleRow
E4NP = ml_dtypes.float8_e4m3
BFNP = ml_dtypes.bfloat16

WH_SCALE = 32.0

# tuning knobs
CROSS = 6            # cross-term MM pairs per (ac, blk): 8=full W correction
XBAR_NUM, XBAR_DEN = 2, 4   # route (XBAR_NUM/XBAR_DEN) of subtiles to xbar
HI_PAT = "adpaadap"  # hi-cast engine per subtile: a=ACT, p=Pool, d=DVE
LO_PAT = "ddpdddpd"  # lo-subtract engine per subtile (DVE or Pool)
CO_PAT = "d"         # PSUM copy-out engine for PE-routed subtiles
EBF_BUFS, NAT_BUFS, ENCT_BUFS = 3, 14, 2
EPS_BUFS, TRANS_BUFS, PF_DEPTH = 2, 3, 2
SMALL_BUFS, CPS_BUFS = 2, 1
XBAR_FLIP = False
EBF_ENG = "d"       # engine for the e_f32->e_bf copy


def build_bass(bloc, s_len, reps=1, coltile=True, dma_only=False):
    """One-core program processing bloc batch rows of length s_len."""
    P = 128
    SB = 512
    n_blk = s_len // SB
    n_hc = H2 // P   # 16 h-chunks
    n_ac = A // P    # 4 a-chunks
    n_wc = H // P    # 8 W_s chunks

    nc = bass.Bass("TRN2", target_bir_lowering=False, debug=False)
    enc = nc.dram_tensor("enc", [bloc, s_len, H2], F32, kind="ExternalInput").ap()
    dec = nc.dram_tensor("dec", [bloc, H], F32, kind="ExternalInput").ap()
    msk = nc.dram_tensor("msk", [bloc, s_len], U8, kind="ExternalInput").ap()
    w_s = nc.dram_tensor("w_s", [H, A], F32, kind="ExternalInput").ap()
    wh8 = nc.dram_tensor("wh8", [P, n_hc, 2, A], U8, kind="ExternalInput").ap()
    wl8 = nc.dram_tensor("wl8", [P, n_hc // 2, 2, A], U8, kind="ExternalInput").ap()
    v_bf = nc.dram_tensor("v_bf", [P, n_ac], U16, kind="ExternalInput").ap()
    id16d = nc.dram_tensor("id16d", [P, P], U16, kind="ExternalInput").ap()
    cone = nc.dram_tensor("cone", [1, 1], F32, kind="ExternalInput").ap()
    ctx_o = nc.dram_tensor("ctx_o", [bloc, P, n_hc], F32, kind="ExternalOutput").ap()
    wgt_o = nc.dram_tensor("wgt_o", [bloc, s_len], F32, kind="ExternalOutput").ap()

    with tile.TileContext(nc) as tc, ExitStack() as ctx:
        consts = ctx.enter_context(tc.tile_pool(name="consts", bufs=1))

        # --- pools for the main loop ---
        encbf_pool = ctx.enter_context(tc.tile_pool(name="encbf", bufs=EBF_BUFS))
        nat16_pool = ctx.enter_context(tc.tile_pool(name="nat16", bufs=NAT_BUFS))
        encT_pool = ctx.enter_context(tc.tile_pool(name="encT", bufs=ENCT_BUFS))

        def load_blk(b, blk):
            # SWDGE cast-DMA: f32 HBM -> bf16 SBUF, one op per block.
            ebf = encbf_pool.tile([P, SB // P, H2], BF16, tag="ebf")
            s0 = blk * SB
            nc.gpsimd.dma_start(
                ebf[:], enc[b, s0 : s0 + SB, :].rearrange("(q p) h -> p q h", p=P)
            )
            return ebf

        def eng_of(ch):
            return {"d": nc.vector, "a": nc.scalar, "p": nc.gpsimd}[ch]

        # prefetch first block, then the hot constants, then block 1
        prefetched = {}
        prefetched[(0, 0)] = load_blk(0, 0)

        # --- constants ---
        whsb = consts.tile([P, n_hc, 2, A], FP8)
        nc.sync.dma_start(whsb[:], wh8.bitcast(FP8))
        wlsb = consts.tile([P, n_hc // 2, 2, A], FP8)
        nc.sync.dma_start(wlsb[:], wl8.bitcast(FP8))
        if n_blk > 1:
            prefetched[(0, 1)] = load_blk(0, 1)
        vsb = consts.tile([P, n_ac], BF16)
        nc.sync.dma_start(vsb[:], v_bf.bitcast(BF16))
        id16 = consts.tile([P, P], BF16)
        nc.sync.dma_start(id16[:], id16d.bitcast(BF16))
        one_bf = consts.tile([1, 1], BF16)
        nc.gpsimd.dma_start(one_bf[:], cone[:])
        ones_f = consts.tile([1, P], F32)
        nc.gpsimd.memset(ones_f[:], 1.0)
        zcol = consts.tile([1, P], BF16)
        nc.gpsimd.memset(zcol[:], 0.0)
        zrow = consts.tile([1, n_hc], BF16)
        nc.gpsimd.memset(zrow[:], 0.0)
        m_all = consts.tile([1, bloc * s_len], U8)
        nc.sync.dma_start(m_all[:], msk.rearrange("b s -> (b s)"))

        # --- dec_projT[a, b] = sum_h W_s[h, a] * dec[b, h] (f32, exact) ---
        dpt = consts.tile([P, n_ac, bloc], F32)
        with tc.tile_pool(name="setup", bufs=1) as setup, tc.tile_pool(
            name="setup_ps", bufs=1, space="PSUM"
        ) as setup_ps:
            wssb = setup.tile([P, n_wc, A], F32)
            nc.sync.dma_start(wssb[:], w_s.rearrange("(c p) a -> p c a", p=P))
            dect = setup.tile([P, n_wc, bloc], F32)
            for c in range(n_wc):
                nc.sync.dma_start(
                    dect[:, c, :],
                    dec[:, c * P : (c + 1) * P].rearrange("b p -> p b"),
                )
            for ca in range(n_ac):
                dps = setup_ps.tile([P, bloc], F32, tag="dp_ps")
                for c in range(n_wc):
                    nc.tensor.matmul(
                        dps[:],
                        wssb[:, c, ca * P : (ca + 1) * P],
                        dect[:, c, :],
                        start=(c == 0),
                        stop=(c == n_wc - 1),
                    )
                nc.scalar.copy(dpt[:, ca, :], dps[:])

        # --- more pools ---
        et_pool = ctx.enter_context(tc.tile_pool(name="et", bufs=3))
        row_pool = ctx.enter_context(tc.tile_pool(name="row", bufs=2))
        etr_pool = ctx.enter_context(tc.tile_pool(name="etr", bufs=2))
        out_pool = ctx.enter_context(tc.tile_pool(name="outp", bufs=2))
        wout_pool = ctx.enter_context(tc.tile_pool(name="wout", bufs=1))
        energy_ps = ctx.enter_context(
            tc.tile_pool(name="energy_ps", bufs=EPS_BUFS, space="PSUM")
        )
        trans_ps = ctx.enter_context(
            tc.tile_pool(name="trans_ps", bufs=TRANS_BUFS, space="PSUM")
        )
        ctx_psp = ctx.enter_context(tc.tile_pool(name="ctx_ps", bufs=CPS_BUFS, space="PSUM"))
        small_ps = ctx.enter_context(
            tc.tile_pool(name="small_ps", bufs=SMALL_BUFS, space="PSUM")
        )

        n_q = SB // P
        total = bloc * n_blk * reps

        def b_of(g):
            return (g // n_blk) % bloc

        # per-block state passed between pipeline stages
        st = {}
        batch = {}

        def stage_load(g):
            if g >= total:
                return
            st[g] = {"ebf": load_blk(b_of(g), g % n_blk)}

        def stage_cast(g):
            nonlocal tctr
            if g >= total or g < 0:
                return
            sg = st[g]
            ebf = sg["ebf"]
            nats = []
            for q in range(n_q):
                nat16 = nat16_pool.tile([P, H2], BF16, tag="nat16")
                n8 = nat16.bitcast(FP8).rearrange("s (h j) -> s h j", j=2)
                hi_e = eng_of(HI_PAT[(tctr + q) % len(HI_PAT)])
                lo_e = eng_of(LO_PAT[(tctr + q) % len(LO_PAT)])
                if hasattr(hi_e, "tensor_copy"):
                    hi_e.tensor_copy(n8[:, :, 0], ebf[:, q, :])
                else:
                    hi_e.copy(n8[:, :, 0], ebf[:, q, :])
                lo_e.tensor_tensor(
                    n8[:, :, 1], ebf[:, q, :], n8[:, :, 0], ALU.subtract
                )
                nats.append(nat16)
            sg["nats"] = nats
            del sg["ebf"]

        def stage_trans(g):
            nonlocal tctr
            if g >= total or g < 0:
                return
            sg = st[g]
            encT16 = encT_pool.tile([P, n_hc, SB], BF16, tag="encT16")
            for q in range(n_q):
                nat16 = sg["nats"][q]
                if ((tctr % XBAR_DEN) < XBAR_NUM) != (XBAR_FLIP and (tctr // 4) % 2 == 1):
                    nc.sync.dma_start_transpose(
                        encT16[:, :, q * P : (q + 1) * P], nat16[:]
                    )
                else:
                    for half in range(2):
                        pt = trans_ps.tile([P, n_hc // 2, P], BF16, tag="pt")
                        for c8 in range(n_hc // 2):
                            c = half * (n_hc // 2) + c8
                            nc.tensor.transpose(
                                pt[:, c8, :],
                                nat16[:, c * P : (c + 1) * P],
                                id16[:],
                            )
                        co_e = eng_of(CO_PAT[(2 * tctr + half) % len(CO_PAT)])
                        dst = encT16[
                            :,
                            half * (n_hc // 2) : (half + 1) * (n_hc // 2),
                            q * P : (q + 1) * P,
                        ]
                        if hasattr(co_e, "tensor_copy"):
                            co_e.tensor_copy(dst, pt[:])
                        else:
                            co_e.copy(dst, pt[:])
                tctr += 1
            sg["encT16"] = encT16

        def batch_state(b):
            if b not in batch:
                e_f32 = row_pool.tile([1, s_len], F32, tag="e_f32")
                esum_p = row_pool.tile([1, n_blk], F32, tag="esum_p")
                etr_sb = etr_pool.tile([P, n_blk * n_q], BF16, tag="etr_sb")
                cps = ctx_psp.tile([P, n_hc], F32, tag="cps")
                nc.tensor.matmul(
                    cps[:], zcol[:], zrow[:], start=True, stop=False,
                    skip_group_check=True,
                )
                batch[b] = {
                    "e_f32": e_f32,
                    "esum_p": esum_p,
                    "etr_sb": etr_sb,
                    "cps": cps,
                }
            return batch[b]

        def stage_compute(g):
            if g >= total or g < 0:
                return
            sg = st[g]
            b, blk = b_of(g), g % n_blk
            bs = batch_state(b)
            encT8 = sg["encT16"].bitcast(FP8).rearrange(
                "p c (s j) -> p c j s", j=2
            )
            sps = small_ps.tile([1, SB], F32, tag="small")
            sg["sps"] = sps
            et_tiles = [None] * n_ac
            # interleave energy chains with scores MMs: e0 e1 s0 e2 s1 e3 s2 s3
            order = [(0, None), (1, None), (None, 0), (2, None), (None, 1),
                     (3, None), (None, 2), (None, 3)]
            for eac, sac in order:
                if eac is not None:
                    ac = eac
                    eps = energy_ps.tile([P, SB], F32, tag="eps")
                    asl = slice(ac * P, (ac + 1) * P)
                    for c in range(n_hc):
                        nc.tensor.matmul(
                            eps[:],
                            whsb[:, c, :, asl],
                            encT8[:, c, :, :],
                            start=(c == 0),
                            stop=(c == n_hc - 1 and CROSS == 0),
                            perf_mode=DR,
                        )
                    for c2 in range(CROSS):
                        nc.tensor.matmul(
                            eps[:],
                            wlsb[:, c2, :, asl],
                            encT8[:, 2 * c2 : 2 * c2 + 2, 0, :],
                            start=False,
                            stop=(c2 == CROSS - 1),
                            perf_mode=DR,
                        )
                    et = et_pool.tile([P, SB], BF16, tag="et")
                    et_tiles[ac] = et
                    nc.scalar.activation(
                        et[:],
                        eps[:],
                        AF.Tanh,
                        bias=dpt[:, ac, b : b + 1],
                        scale=1.0 / WH_SCALE,
                    )
                else:
                    ac = sac
                    nc.tensor.matmul(
                        sps[:],
                        vsb[:, ac : ac + 1],
                        et_tiles[ac][:],
                        start=(ac == 0),
                        stop=(ac == n_ac - 1),
                    )
            esl = slice(blk * SB, (blk + 1) * SB)
            e_f32 = bs["e_f32"]
            nc.scalar.activation(e_f32[0:1, esl], sps[:], AF.Exp)
            del sg["encT16"]

        def stage_tail(g):
            if g >= total or g < 0:
                return
            sg = st.pop(g)
            b, blk = b_of(g), g % n_blk
            bs = batch_state(b)
            esl = slice(blk * SB, (blk + 1) * SB)
            e_f32 = bs["e_f32"]
            nc.vector.tensor_mul(
                e_f32[0:1, esl],
                e_f32[0:1, esl],
                m_all[0:1, b * s_len + blk * SB : b * s_len + (blk + 1) * SB],
            )
            nc.vector.reduce_sum(
                bs["esum_p"][0:1, blk : blk + 1],
                e_f32[0:1, esl],
                axis=mybir.AxisListType.X,
            )
            e_bf = et_pool.tile([1, SB], BF16, tag="e_bf")
            if EBF_ENG == "a":
                nc.scalar.copy(e_bf[:], e_f32[0:1, esl])
            else:
                nc.vector.tensor_copy(e_bf[:], e_f32[0:1, esl])
            etr = small_ps.tile([P, n_q], F32, tag="small")
            for q in range(n_q):
                nc.tensor.matmul(
                    etr[:, q : q + 1],
                    e_bf[0:1, q * P : (q + 1) * P],
                    one_bf[:],
                    start=True,
                    stop=True,
                )
            tsl = slice(blk * n_q, (blk + 1) * n_q)
            etr_sb = bs["etr_sb"]
            nc.scalar.copy(etr_sb[:, tsl], etr[:])
            cps = bs["cps"]
            for q in range(n_q):
                n8q = sg["nats"][q].bitcast(FP8).rearrange(
                    "s (h j) -> s h j", j=2
                )
                ecol = etr_sb[:, blk * n_q + q : blk * n_q + q + 1]
                for c in range(n_hc):
                    for j in range(2):
                        nc.tensor.matmul(
                            cps[:, c : c + 1],
                            n8q[:, c * P : (c + 1) * P, j],
                            ecol,
                            start=False,
                            stop=(
                                blk == n_blk - 1 and q == n_q - 1 and j == 1
                            ),
                            skip_group_check=True,
                        )
            if blk == n_blk - 1:
                bst = batch.pop(b)
                esum = row_pool.tile([1, 1], F32, tag="esum")
                nc.vector.reduce_sum(
                    esum[:], bst["esum_p"][:], axis=mybir.AxisListType.X
                )
                inv = row_pool.tile([1, 1], F32, tag="inv")
                nc.vector.reciprocal(inv[:], esum[:])
                invp = small_ps.tile([P, 1], F32, tag="small")
                nc.tensor.matmul(
                    invp[:], ones_f[:], inv[:], start=True, stop=True
                )
                inv_all = row_pool.tile([P, 1], F32, tag="inv_all")
                nc.scalar.copy(inv_all[:], invp[:])
                ctx_sb = out_pool.tile([P, n_hc], F32, tag="ctx_sb")
                nc.scalar.activation(
                    ctx_sb[:], bst["cps"][:], AF.Copy, scale=inv_all[:]
                )
                wgt_sb = wout_pool.tile([1, s_len], F32, tag="wgt_sb")
                nc.scalar.activation(
                    wgt_sb[:], bst["e_f32"][:], AF.Copy, scale=inv[:]
                )
                nc.sync.dma_start(ctx_o[b], ctx_sb[:])
                nc.sync.dma_start(wgt_o[b : b + 1, :], wgt_sb[:])

        tctr = 0
        PF = PF_DEPTH
        # software-pipelined main loop
        for b0, blk0 in list(prefetched.keys()):
            st[b0 * n_blk + blk0] = {"ebf": prefetched[(b0, blk0)]}
        for g in range(total + 2):
            gl = g + PF
            if gl < total and gl not in st:
                stage_load(gl)
            stage_cast(g)
            stage_compute(g - 1)
            stage_tail(g - 2)
            stage_trans(g)

    return _spill_excess_waits(nc)


class _Runner:
    """Compile once, execute many times with device-resident inputs."""

    def __init__(self, bloc, s_len, n_cores=N_CORES):
        import jax
        from jax.experimental.shard_map import shard_map
        from jax.sharding import Mesh, PartitionSpec

        from concourse import bass2jax

        bass2jax.install_neuronx_cc_hook()
        self.n_cores = n_cores
        self.bloc = bloc
        nc = build_bass(bloc, s_len)
        in_names, out_names, out_avals = [], [], []
        for alloc in nc.m.functions[0].allocations:
            if not isinstance(alloc, mybir.MemoryLocationSet):
                continue
            name = alloc.memorylocations[0].name
            if alloc.kind == "ExternalInput":
                in_names.append(name)
            elif alloc.kind == "ExternalOutput":
                out_names.append(name)
                out_avals.append(
                    jax.core.ShapedArray(
                        tuple(alloc.tensor_shape), mybir.dt.np(alloc.dtype)
                    )
                )
        partition_name = (
            nc.partition_id_tensor.name if nc.partition_id_tensor else None
        )
        if partition_name is not None:
            in_names = [n for n in in_names if n != partition_name]
        self.in_names = in_names
        self.out_names = out_names
        self.out_avals = out_avals
        n_params = len(in_names)
        n_outs = len(out_names)
        all_in_names = tuple(in_names) + tuple(out_names)
        if partition_name is not None:
            all_in_names = all_in_names + (partition_name,)

        def _body(*args):
            operands = list(args)
            if partition_name is not None:
                operands.append(bass2jax.partition_id_tensor())
            outs = bass2jax._bass_exec_p.bind(
                *operands,
                out_avals=tuple(out_avals),
                in_names=all_in_names,
                out_names=tuple(out_names),
                lowering_input_output_aliases=(),
                sim_require_finite=True,
                sim_require_nnan=True,
                nc=nc,
            )
            return tuple(outs)

        devices = jax.devices()[:n_cores]
        self.mesh = Mesh(np.asarray(devices), ("core",))
        in_specs = (PartitionSpec("core"),) * (n_params + n_outs)
        out_specs = (PartitionSpec("core"),) * n_outs
        self.sharded = jax.jit(
            shard_map(
                _body,
                mesh=self.mesh,
                in_specs=in_specs,
                out_specs=out_specs,
                check_rep=False,
            ),
            donate_argnums=tuple(range(n_params, n_params + n_outs)),
            keep_unused=True,
        )
        self._jax = jax

    def put_inputs(self, per_core_maps):
        import jax
        from jax.sharding import NamedSharding, PartitionSpec

        sh = NamedSharding(self.mesh, PartitionSpec("core"))
        arrs = []
        for name in self.in_names:
            cat = np.concatenate(
                [np.asarray(m[name]) for m in per_core_maps], axis=0
            )
            arrs.append(jax.device_put(cat, sh))
        jax.block_until_ready(arrs)
        return arrs

    def _zero_outs(self):
        return [
            np.zeros((self.n_cores * a.shape[0], *a.shape[1:]), a.dtype)
            for a in self.out_avals
        ]

    def run(self, dev_inputs):
        outs = self.sharded(*dev_inputs, *self._zero_outs())
        self._jax.block_until_ready(outs)
        return outs

    def run_np(self, dev_inputs):
        outs = self.run(dev_inputs)
        return {n: np.asarray(o) for n, o in zip(self.out_names, outs)}


_RUNNER_CACHE = {}


def _get_runner(bloc, s_len, n_cores=N_CORES):
    key = (bloc, s_len, n_cores)
    if key not in _RUNNER_CACHE:
        _RUNNER_CACHE[key] = _Runner(bloc, s_len, n_cores)
    return _RUNNER_CACHE[key]


def _prep_weights(W_h):
    """Host prep: Whi/Wlo fp8 tiles (as uint8 views for device_put)."""
    P, n_hc = 128, H2 // 128
    A_ = A
    Asc = np.ascontiguousarray(W_h, dtype=np.float32) * WH_SCALE
    Whi = Asc.astype(E4NP)
    R = Asc - Whi.astype(np.float32)
    Wlo = R.astype(E4NP)
    wh = np.zeros((P, n_hc, 2, A_), E4NP)
    wl = np.zeros((P, n_hc // 2, 2, A_), E4NP)
    for c in range(n_hc):
        blkw = Whi[c * P : (c + 1) * P, :]
        wh[:, c, 0, :] = blkw
        wh[:, c, 1, :] = blkw
    for c2 in range(n_hc // 2):
        for j in range(2):
            wl[:, c2, j, :] = Wlo[(2 * c2 + j) * P : (2 * c2 + j + 1) * P, :]
    return wh.view(np.uint8), wl.view(np.uint8)


def make_in_maps(decoder_hidden, encoder_outputs, mask, W_s, W_h, v, n_cores=N_CORES):
    b_full = mask.shape[0]
    bloc = b_full // n_cores
    enc_np = np.ascontiguousarray(encoder_outputs, dtype=np.float32)
    dec_np = np.ascontiguousarray(decoder_hidden, dtype=np.float32)
    msk_np = np.ascontiguousarray(mask).view(np.uint8)
    ws_np = np.ascontiguousarray(W_s, dtype=np.float32)
    wh_np, wl_np = _prep_weights(W_h)
    v_np = np.ascontiguousarray(v, dtype=np.float32)
    vsb = np.zeros((128, A // 128), BFNP)
    for k in range(A // 128):
        vsb[:, k] = v_np[k * 128 : (k + 1) * 128].astype(BFNP)
    vsb_u16 = vsb.view(np.uint16)
    id_np = np.eye(128).astype(BFNP).view(np.uint16)
    one_np = np.ones((1, 1), np.float32)
    in_maps = []
    for i in range(n_cores):
        sl = slice(i * bloc, (i + 1) * bloc)
        in_maps.append(
            {
                "enc": enc_np[sl],
                "dec": dec_np[sl],
                "msk": msk_np[sl],
                "w_s": ws_np,
                "wh8": wh_np,
                "wl8": wl_np,
                "v_bf": vsb_u16,
                "id16d": id_np,
                "cone": one_np,
            }
        )
    return in_maps


def run_sharded(decoder_hidden, encoder_outputs, mask, W_s, W_h, v, n_cores=N_CORES):
    b_full, s_len = mask.shape
    bloc = b_full // n_cores
    runner = _get_runner(bloc, s_len, n_cores)
    in_maps = make_in_maps(
        decoder_hidden, encoder_outputs, mask, W_s, W_h, v, n_cores
    )
    dev_in = runner.put_inputs(in_maps)
    outs = runner.run_np(dev_in)
    ctxT = outs["ctx_o"].reshape(b_full, 128, H2 // 128)
    ctx = ctxT.transpose(0, 2, 1).reshape(b_full, H2)
    wgt = outs["wgt_o"].reshape(b_full, s_len)
    return ctx, wgt


def kernel(decoder_hidden, encoder_outputs, mask, W_s, W_h, v):
    decoder_hidden = np.asarray(decoder_hidden)
    encoder_outputs = np.asarray(encoder_outputs)
    mask = np.asarray(mask)
    W_s = np.asarray(W_s)
    W_h = np.asarray(W_h)
    v = np.asarray(v)
    ctx, wgt = run_sharded(decoder_hidden, encoder_outputs, mask, W_s, W_h, v)
    return ctx, wgt
